# revision 46
# baseline (speedup 1.0000x reference)
"""Cross-attention kernel for one TRN2 chip (8 NeuronCores).

Sharding: core = (batch b in {0,1}) x (head-group of 4 heads).  Each core
computes attention for its 4 heads of its batch element and a partial output
projection [N, 1024]; the host sums the 4 partials per batch and adds bias.

Key structure (all matmuls bf16, fp32 PSUM):
  - x/ctx cast-loaded to bf16 (SWDGE), transposed on the PE via regular
    matmuls against a bf16 identity (keeps HAM warm; ~2x faster than
    transpose-mode).
  - QK per m-tile: two concurrent row-tiled matmuls (head s0 on array rows
    0-63, s1 on rows 64-127) into one [128,1024] PSUM tile; ONE wide exp
    [128,1024] on ScalarE covers both heads; AV accumulates [65,512] per
    head with a ones-column producing the softmax denominator for free.
  - PSUM: sT double-buffered (4 banks) + oT double-buffered (4 banks) so
    the PE never stalls on the activation and HAM stays at 2.4 GHz.
  - Normalization: denominators go PSUM->SBUF->DRAM, are gathered into a
    [128,64] tile, reciprocal_approx_accurate, scattered back, broadcast-
    DMA'd across partitions, one tensor_mul per d-chunk.
"""

import numpy as np

import concourse.bass as bass
import concourse.mybir as mybir
import concourse.tile as tile
from concourse import bacc
from concourse.masks import make_identity
from concourse.bass import ts
from concourse.bass_utils import run_bass_kernel_spmd
B, N, M, C = 2, 2048, 2048, 1024
HEADS, DH = 16, 64
H_PER = 4                # heads per core
DHC = H_PER * DH         # 256: per-core slice of INNER
SCALE = DH ** -0.5
P = 128
NT = N // P              # 16 n-tiles
MT = M // P              # 16 m-tiles
CCH = C // P             # 8 contraction chunks
FD = 512                 # attention n-chunk (PSUM bank)
NJ = N // FD             # 4 n-chunks
N_CORES = 8

F32 = mybir.dt.float32
BF16 = mybir.dt.bfloat16
I32 = mybir.dt.int32
EXP = mybir.ActivationFunctionType.Exp
# Schraudolph fast-exp constants: exp(x*SCALE) ~= bitcast(i32(x*KS + BS))
KS = SCALE * (1 << 23) / float(np.log(2.0))
BS = float(127 * (1 << 23) - 366392)

_CACHE = {}
DEBUG_PROBES = False


def _build():
    nc = bacc.Bacc("TRN2", target_bir_lowering=False, debug=False,
                   num_devices=N_CORES, num_swdge_queues=4)

    xT_d = nc.dram_tensor("xT", (C, N), BF16, kind="ExternalInput").ap()
    cT_d = nc.dram_tensor("cT", (C, M), BF16, kind="ExternalInput").ap()
    msk_d = nc.dram_tensor("msk", (M, 1), F32, kind="ExternalInput").ap()
    wq_d = nc.dram_tensor("wq", (C, DHC), BF16, kind="ExternalInput").ap()
    wk_d = nc.dram_tensor("wk", (C, DHC), BF16, kind="ExternalInput").ap()
    wv_d = nc.dram_tensor("wv", (C, DHC), BF16, kind="ExternalInput").ap()
    wo_d = nc.dram_tensor("wo", (DHC, C), BF16, kind="ExternalInput").ap()
    y_d = nc.dram_tensor("y", (N, C), BF16, kind="ExternalOutput").ap()

    with tile.TileContext(nc) as tc:
        with (
            tc.tile_pool(name="const", bufs=1) as const,
            tc.tile_pool(name="stage", bufs=1) as stage,
            tc.tile_pool(name="pTp", bufs=3) as pTp,
            tc.tile_pool(name="i32p", bufs=2) as i32p,
            tc.tile_pool(name="drn", bufs=2) as drn,
            tc.tile_pool(name="norm", bufs=4) as norm,
            tc.tile_pool(name="rbp", bufs=2) as rbp,
            tc.tile_pool(name="yp", bufs=3) as yp,
            tc.tile_pool(name="dramp", bufs=1, space="DRAM") as dramp,
        ):
            # ---- persistent SBUF tensors ----
            xT = const.tile([P, CCH, N], BF16, name="xT")
            cT = const.tile([P, CCH, M], BF16, name="cT")
            qT2 = const.tile([P, 2, N], BF16, name="qT2")
            kT2 = const.tile([P, 2, M], BF16, name="kT2")
            # v: [m-partition, m-tile, head, d(64)+ones(1)]
            v_sb = const.tile([P, MT, H_PER, DH + 1], BF16, name="v")
            wq_sb = const.tile([P, CCH, DHC], BF16, name="wq")
            wk_sb = const.tile([P, CCH, DHC], BF16, name="wk")
            wv_sb = const.tile([P, CCH, DHC], BF16, name="wv")
            wo_sb = const.tile([P, 2, C], BF16, name="wo")
            msk_sb = const.tile([P, MT, 1], F32, name="msk")
            oTn = const.tile([P, 2, N], BF16, name="oTn")

            # ---- weights + mask (pre-cast to bf16 on host) ----
            nc.sync.dma_start(
                out=wk_sb, in_=wk_d.rearrange("(cc p) d -> p cc d", p=P))
            nc.sync.dma_start(
                out=wv_sb, in_=wv_d.rearrange("(cc p) d -> p cc d", p=P))
            nc.sync.dma_start(
                out=wq_sb, in_=wq_d.rearrange("(cc p) d -> p cc d", p=P))
            nc.sync.dma_start(
                out=wo_sb, in_=wo_d.rearrange("(dc p) e -> p dc e", p=P))
            nc.sync.dma_start(
                out=msk_sb, in_=msk_d.rearrange("(t p) o -> p t o", p=P))

            nc.vector.memset(v_sb, 1.0)
            identf = stage.tile([P, P], F32, name="identf")
            make_identity(nc, identf)
            identb = const.tile([P, P], BF16, name="identb")
            nc.vector.tensor_copy(identb, identf)

            ps_p_cm = tc.tile_pool(name="ps_p", bufs=3, space="PSUM")
            ps_p = ps_p_cm.__enter__()       # [128,512] projections: 3 banks

            # project one n/m-chunk j of q or k (both d-chunks dc)
            def proj_T(w_sb, srcT, dstT2, dc, j, alt):
                ps = ps_p.tile([P, FD], F32, name="kq")
                for cc in range(CCH):
                    nc.tensor.matmul(
                        ps, lhsT=w_sb[:, cc, ts(dc, P)],
                        rhs=srcT[:, cc, ts(j, FD)],
                        start=(cc == 0), stop=(cc == CCH - 1))
                dst = dstT2[:, dc, ts(j, FD)]
                if alt:
                    nc.vector.tensor_copy(dst, ps)
                else:
                    nc.scalar.copy(dst, ps)

            # V projection for two m-tiles (one [128,512] PSUM tile)
            def proj_V(m0):
                vp = ps_p.tile([P, 2, DHC], F32, name="vp")
                for mi in range(2):
                    for cc in range(CCH):
                        nc.tensor.matmul(
                            vp[:, mi, :],
                            lhsT=cT[:, cc, ts(m0 + mi, P)],
                            rhs=wv_sb[:, cc, :],
                            start=(cc == 0), stop=(cc == CCH - 1))
                nc.vector.tensor_copy(
                    v_sb[:, m0:m0 + 2, :, 0:DH],
                    vp.rearrange("p mi (h d) -> p mi h d", h=H_PER))
                for mi in range(2):
                    nc.vector.tensor_scalar_mul(
                        v_sb[:, m0 + mi, :, :], v_sb[:, m0 + mi, :, :],
                        msk_sb[:, m0 + mi, :])

            # ---- phase A: xbar-transpose ctx/x from DRAM, project K/V/Q ----
            alt = 0
            cTv = cT_d.rearrange("(cc p) n -> p cc n", p=P)
            xTv = xT_d.rearrange("(cc p) n -> p cc n", p=P)
            for h in range(2):
                for cc in range(CCH):
                    nc.sync.dma_start(
                        out=cT[:, cc, ts(h, N // 2)],
                        in_=cTv[:, cc, ts(h, N // 2)])
                for cc in range(CCH):
                    nc.sync.dma_start(
                        out=xT[:, cc, ts(h, N // 2)],
                        in_=xTv[:, cc, ts(h, N // 2)])
            for g in range(4):
                for dc in range(2):
                    proj_T(wk_sb, cT, kT2, dc, g, alt % 2)
                    alt += 1
                proj_V(4 * g)
                proj_V(4 * g + 2)
            for g in range(4):
                for dc in range(2):
                    proj_T(wq_sb, xT, qT2, dc, g, alt % 2)
                    alt += 1

            ps_p_cm.__exit__(None, None, None)

            # ---- phase B: attention (o accumulated in natural [n, d]
            # layout so the softmax denominator is a per-partition scalar) ----
            ps_s_cm = tc.tile_pool(name="ps_s", bufs=2, space="PSUM")
            ps_s = ps_s_cm.__enter__()       # [128,1024] scores: 4 banks
            ps_o_cm = tc.tile_pool(name="ps_o", bufs=1, space="PSUM")
            ps_o = ps_o_cm.__enter__()       # 2x[128,260] per j: 2 banks

            def qk(sT, dc, j, m):
                for s in range(2):
                    nc.tensor.matmul(
                        sT[:, s, :],
                        lhsT=kT2[s * DH:(s + 1) * DH, dc, ts(m, P)],
                        rhs=qT2[s * DH:(s + 1) * DH, dc, ts(j, FD)],
                        start=True, stop=True)

            def av(oPs, pT, dc, m):
                # o_nat[n, d] += pT[m, n]^T @ v[m, d|1]; stationary = pT chunk
                for s in range(2):
                    for sub in range(4):
                        nc.tensor.matmul(
                            oPs[s][:, sub, :],
                            lhsT=pT[:, s, ts(sub, P)],
                            rhs=v_sb[:, m, 2 * dc + s, :],
                            start=(m == 0 and sub == 0),
                            stop=(m == MT - 1),
                            skip_group_check=True)

            for dc in range(2):
                for j in range(NJ):
                    oPs = [ps_o.tile([P, 4, DH + 1], F32, name=f"o{s}")
                           for s in range(2)]
                    pTs = []
                    for m in range(MT):
                        sT = ps_s.tile([P, 2, FD], F32, name="sT")
                        qk(sT, dc, j, m)
                        if m >= 2:
                            av(oPs, pTs[m - 2], dc, m - 2)
                        pT = pTp.tile([P, 2, FD], BF16, name="pT")
                        if m % 3 == 2:
                            # DVE fast-exp (Schraudolph) to offload ScalarE
                            it = i32p.tile([P, 2, FD], I32, name="it")
                            nc.vector.tensor_scalar(
                                it, sT, KS, BS,
                                op0=mybir.AluOpType.mult,
                                op1=mybir.AluOpType.add)
                            nc.vector.tensor_copy(pT, it.bitcast(F32))
                        else:
                            nc.scalar.activation(pT, sT, EXP, scale=SCALE)
                        pTs.append(pT)
                    av(oPs, pTs[MT - 2], dc, MT - 2)
                    av(oPs, pTs[MT - 1], dc, MT - 1)
                    # drain + normalize: D is column 64 of each (s, sub) block
                    o_sb = drn.tile([P, 2, 4, DH + 1], F32, name="o_sb")
                    for s in range(2):
                        nc.vector.tensor_copy(o_sb[:, s], oPs[s])
                    rc = norm.tile([P, 2, 4, 1], F32, name="rc")
                    nc.vector.reciprocal(rc, o_sb[:, :, :, DH:DH + 1])
                    o_bf = drn.tile([P, 4, 2, DH], BF16, name="o_bf")
                    for s in range(2):
                        for sub in range(4):
                            nc.vector.tensor_scalar_mul(
                                o_bf[:, sub, s, :], o_sb[:, s, sub, 0:DH],
                                rc[:, s, sub, :])
                    # transpose back to d-major for the output projection
                    if dc == 1 and j == NJ - 1:
                        # last group: deferred PE transpose (runs after the
                        # already-ready y tiles so the PE queue never stalls)
                        last_obf = o_bf
                    else:
                        for sub in range(4):
                            nc.sync.dma_start_transpose(
                                out=oTn[:, dc, j * FD + sub * P:
                                        j * FD + (sub + 1) * P],
                                in_=o_bf[:, sub].rearrange("p s d -> p (s d)"))

            ps_o_cm.__exit__(None, None, None)
            ps_s_cm.__exit__(None, None, None)

            # ---- phase C: output projection ----
            ps_y_cm = tc.tile_pool(name="ps_y", bufs=3, space="PSUM")
            ps_y = ps_y_cm.__enter__()

            def y_tile(i):
                y_ps = ps_y.tile([P, C], F32, name="y")
                for col in range(2):
                    for dc in range(2):
                        nc.tensor.matmul(
                            y_ps[:, ts(col, FD)],
                            lhsT=oTn[:, dc, ts(i, P)],
                            rhs=wo_sb[:, dc, ts(col, FD)],
                            start=(dc == 0), stop=(dc == 1))
                y_sb = yp.tile([P, C], BF16, name="ysb")
                nc.vector.tensor_copy(y_sb[:, 0:FD], y_ps[:, 0:FD])
                nc.scalar.copy(y_sb[:, FD:C], y_ps[:, FD:C])
                nc.sync.dma_start(out=y_d[ts(i, P), :], in_=y_sb)

            for i in range(NT - 4):
                y_tile(i)
            ps_t2_cm = tc.tile_pool(name="ps_t2", bufs=1, space="PSUM")
            ps_t2 = ps_t2_cm.__enter__()
            tp = ps_t2.tile([P, FD], F32, name="tpy")
            for sub in range(4):
                nc.tensor.matmul(
                    tp[:, ts(sub, P)],
                    lhsT=last_obf[:, sub].rearrange("p s d -> p (s d)"),
                    rhs=identb, start=True, stop=True)
            nc.vector.tensor_copy(oTn[:, 1, ts(NJ - 1, FD)], tp)
            for i in range(NT - 4, NT):
                y_tile(i)
            ps_t2_cm.__exit__(None, None, None)
            ps_y_cm.__exit__(None, None, None)

    nc.compile()
    return nc


def _in_maps(x, context, mask, Wq, Wk, Wv, Wo):
    from ml_dtypes import bfloat16
    maps = []
    xb = np.asarray(x, dtype=np.float32).astype(bfloat16)
    cb = np.asarray(context, dtype=np.float32).astype(bfloat16)
    for core in range(N_CORES):
        b, hg = core // H_PER, core % H_PER
        c0 = hg * DHC
        maps.append({
            "xT": np.ascontiguousarray(xb[b].T),
            "cT": np.ascontiguousarray(cb[b].T),
            "msk": np.ascontiguousarray(
                np.asarray(mask[b]).astype(np.float32).reshape(M, 1)),
            "wq": np.ascontiguousarray(
                np.asarray(Wq[:, c0:c0 + DHC], dtype=np.float32)
                .astype(bfloat16)),
            "wk": np.ascontiguousarray(
                np.asarray(Wk[:, c0:c0 + DHC], dtype=np.float32)
                .astype(bfloat16)),
            "wv": np.ascontiguousarray(
                np.asarray(Wv[:, c0:c0 + DHC], dtype=np.float32)
                .astype(bfloat16)),
            "wo": np.ascontiguousarray(
                np.asarray(Wo[c0:c0 + DHC, :], dtype=np.float32)
                .astype(bfloat16)),
        })
    return maps


def _gather(results, bo):
    out = np.zeros((B, N, C), dtype=np.float32)
    for core in range(N_CORES):
        out[core // H_PER] += np.asarray(results[core]["y"],
                                         dtype=np.float32)
    out += np.asarray(bo, dtype=np.float32)
    return out


def kernel(x, context, mask, Wq, Wk, Wv, Wo, bo, **extra_kwargs):
    if "nc" not in _CACHE:
        _CACHE["nc"] = _build()
    nc = _CACHE["nc"]
    maps = _in_maps(x, context, mask, Wq, Wk, Wv, Wo)
    res = run_bass_kernel_spmd(nc, maps, core_ids=list(range(N_CORES)),
                               **extra_kwargs)
    out = _gather(res.results, bo)
    if extra_kwargs:
        _CACHE["last_result"] = res
    return out


# revision 47
# speedup vs baseline: 1.3242x; 1.3242x over previous
"""Cross-attention kernel for one TRN2 chip (8 NeuronCores).

Sharding: core = (batch b in {0,1}) x (head-group of 4 heads).  Each core
computes attention for its 4 heads of its batch element and a partial output
projection [N, 1024]; the host sums the 4 partials per batch and adds bias.

Key structure (all matmuls bf16, fp32 PSUM):
  - x/ctx cast-loaded to bf16 (SWDGE), transposed on the PE via regular
    matmuls against a bf16 identity (keeps HAM warm; ~2x faster than
    transpose-mode).
  - QK per m-tile: two concurrent row-tiled matmuls (head s0 on array rows
    0-63, s1 on rows 64-127) into one [128,1024] PSUM tile; ONE wide exp
    [128,1024] on ScalarE covers both heads; AV accumulates [65,512] per
    head with a ones-column producing the softmax denominator for free.
  - PSUM: sT double-buffered (4 banks) + oT double-buffered (4 banks) so
    the PE never stalls on the activation and HAM stays at 2.4 GHz.
  - Normalization: denominators go PSUM->SBUF->DRAM, are gathered into a
    [128,64] tile, reciprocal_approx_accurate, scattered back, broadcast-
    DMA'd across partitions, one tensor_mul per d-chunk.
"""

import numpy as np

import concourse.bass as bass
import concourse.mybir as mybir
import concourse.tile as tile
from concourse import bacc
from concourse.masks import make_identity
from concourse.bass import ts
from concourse.bass_utils import run_bass_kernel_spmd
B, N, M, C = 2, 2048, 2048, 1024
HEADS, DH = 16, 64
H_PER = 4                # heads per core
DHC = H_PER * DH         # 256: per-core slice of INNER
SCALE = DH ** -0.5
P = 128
NT = N // P              # 16 n-tiles
MT = M // P              # 16 m-tiles
CCH = C // P             # 8 contraction chunks
FD = 512                 # attention n-chunk (PSUM bank)
NJ = N // FD             # 4 n-chunks
N_CORES = 8

F32 = mybir.dt.float32
BF16 = mybir.dt.bfloat16
I32 = mybir.dt.int32
EXP = mybir.ActivationFunctionType.Exp
# Schraudolph fast-exp constants: exp(x*SCALE) ~= bitcast(i32(x*KS + BS))
KS = SCALE * (1 << 23) / float(np.log(2.0))
BS = float(127 * (1 << 23) - 366392)

_CACHE = {}
DEBUG_PROBES = False


def _build():
    nc = bacc.Bacc("TRN2", target_bir_lowering=False, debug=False,
                   num_devices=N_CORES, num_swdge_queues=4)

    xT_d = nc.dram_tensor("xT", (C, N), BF16, kind="ExternalInput").ap()
    cT_d = nc.dram_tensor("cT", (C, M), BF16, kind="ExternalInput").ap()
    msk_d = nc.dram_tensor("msk", (M, 1), F32, kind="ExternalInput").ap()
    wq_d = nc.dram_tensor("wq", (C, DHC), BF16, kind="ExternalInput").ap()
    wk_d = nc.dram_tensor("wk", (C, DHC), BF16, kind="ExternalInput").ap()
    wv_d = nc.dram_tensor("wv", (C, DHC), BF16, kind="ExternalInput").ap()
    wo_d = nc.dram_tensor("wo", (DHC, C), BF16, kind="ExternalInput").ap()
    y_d = nc.dram_tensor("y", (N, C), BF16, kind="ExternalOutput").ap()

    with tile.TileContext(nc) as tc:
        with (
            tc.tile_pool(name="const", bufs=1) as const,
            tc.tile_pool(name="stage", bufs=1) as stage,
            tc.tile_pool(name="pTp", bufs=3) as pTp,
            tc.tile_pool(name="i32p", bufs=2) as i32p,
            tc.tile_pool(name="drn", bufs=2) as drn,
            tc.tile_pool(name="norm", bufs=4) as norm,
            tc.tile_pool(name="rbp", bufs=2) as rbp,
            tc.tile_pool(name="yp", bufs=3) as yp,
            tc.tile_pool(name="dramp", bufs=1, space="DRAM") as dramp,
        ):
            # ---- persistent SBUF tensors ----
            xT = const.tile([P, CCH, N], BF16, name="xT")
            cT = const.tile([P, CCH, M], BF16, name="cT")
            qT2 = const.tile([P, 2, N], BF16, name="qT2")
            kT2 = const.tile([P, 2, M], BF16, name="kT2")
            # v: [m-partition, m-tile, head, d(64)+ones(1)]
            v_sb = const.tile([P, MT, H_PER, DH + 1], BF16, name="v")
            wq_sb = const.tile([P, CCH, DHC], BF16, name="wq")
            wk_sb = const.tile([P, CCH, DHC], BF16, name="wk")
            wv_sb = const.tile([P, CCH, DHC], BF16, name="wv")
            wo_sb = const.tile([P, 2, C], BF16, name="wo")
            msk_sb = const.tile([P, MT, 1], F32, name="msk")
            oTn = const.tile([P, 2, N], BF16, name="oTn")

            # ---- weights + mask (pre-cast to bf16 on host) ----
            nc.sync.dma_start(
                out=wk_sb, in_=wk_d.rearrange("(cc p) d -> p cc d", p=P))
            nc.sync.dma_start(
                out=wv_sb, in_=wv_d.rearrange("(cc p) d -> p cc d", p=P))
            nc.sync.dma_start(
                out=wq_sb, in_=wq_d.rearrange("(cc p) d -> p cc d", p=P))
            nc.sync.dma_start(
                out=wo_sb, in_=wo_d.rearrange("(dc p) e -> p dc e", p=P))
            nc.sync.dma_start(
                out=msk_sb, in_=msk_d.rearrange("(t p) o -> p t o", p=P))

            nc.vector.memset(v_sb, 1.0)
            identf = stage.tile([P, P], F32, name="identf")
            make_identity(nc, identf)
            identb = const.tile([P, P], BF16, name="identb")
            nc.vector.tensor_copy(identb, identf)

            ps_p_cm = tc.tile_pool(name="ps_p", bufs=3, space="PSUM")
            ps_p = ps_p_cm.__enter__()       # [128,512] projections: 3 banks

            # project one n/m-chunk j of q or k (both d-chunks dc)
            def proj_T(w_sb, srcT, dstT2, dc, j, alt):
                ps = ps_p.tile([P, FD], F32, name="kq")
                for cc in range(CCH):
                    nc.tensor.matmul(
                        ps, lhsT=w_sb[:, cc, ts(dc, P)],
                        rhs=srcT[:, cc, ts(j, FD)],
                        start=(cc == 0), stop=(cc == CCH - 1))
                dst = dstT2[:, dc, ts(j, FD)]
                if alt:
                    nc.vector.tensor_copy(dst, ps)
                else:
                    nc.scalar.copy(dst, ps)

            # V projection for two m-tiles (one [128,512] PSUM tile)
            def proj_V(m0):
                vp = ps_p.tile([P, 2, DHC], F32, name="vp")
                for mi in range(2):
                    for cc in range(CCH):
                        nc.tensor.matmul(
                            vp[:, mi, :],
                            lhsT=cT[:, cc, ts(m0 + mi, P)],
                            rhs=wv_sb[:, cc, :],
                            start=(cc == 0), stop=(cc == CCH - 1))
                nc.vector.tensor_copy(
                    v_sb[:, m0:m0 + 2, :, 0:DH],
                    vp.rearrange("p mi (h d) -> p mi h d", h=H_PER))
                for mi in range(2):
                    nc.vector.tensor_scalar_mul(
                        v_sb[:, m0 + mi, :, :], v_sb[:, m0 + mi, :, :],
                        msk_sb[:, m0 + mi, :])

            # ---- phase A: xbar-transpose ctx/x from DRAM, project K/V/Q ----
            alt = 0
            cTv = cT_d.rearrange("(cc p) n -> p cc n", p=P)
            xTv = xT_d.rearrange("(cc p) n -> p cc n", p=P)
            for h in range(2):
                for cc in range(CCH):
                    nc.sync.dma_start(
                        out=cT[:, cc, ts(h, N // 2)],
                        in_=cTv[:, cc, ts(h, N // 2)])
                for cc in range(CCH):
                    nc.sync.dma_start(
                        out=xT[:, cc, ts(h, N // 2)],
                        in_=xTv[:, cc, ts(h, N // 2)])
            for g in range(4):
                for dc in range(2):
                    proj_T(wk_sb, cT, kT2, dc, g, alt % 2)
                    alt += 1
                proj_V(4 * g)
                proj_V(4 * g + 2)
            for g in range(4):
                for dc in range(2):
                    proj_T(wq_sb, xT, qT2, dc, g, alt % 2)
                    alt += 1

            ps_p_cm.__exit__(None, None, None)

            # ---- phase B: attention (o accumulated in natural [n, d]
            # layout so the softmax denominator is a per-partition scalar) ----
            ps_s_cm = tc.tile_pool(name="ps_s", bufs=3, space="PSUM")
            ps_s = ps_s_cm.__enter__()       # [128,1024] scores: 6 banks
            ps_o_cm = tc.tile_pool(name="ps_o", bufs=1, space="PSUM")
            ps_o = ps_o_cm.__enter__()       # 2x[128,260] per j: 2 banks

            def qk(sT, dc, j, m):
                for s in range(2):
                    nc.tensor.matmul(
                        sT[:, s, :],
                        lhsT=kT2[s * DH:(s + 1) * DH, dc, ts(m, P)],
                        rhs=qT2[s * DH:(s + 1) * DH, dc, ts(j, FD)],
                        start=True, stop=True)

            def av(oPs, pT, dc, m):
                # o_nat[n, d] += pT[m, n]^T @ v[m, d|1]; stationary = pT chunk
                for s in range(2):
                    for sub in range(4):
                        nc.tensor.matmul(
                            oPs[s][:, sub, :],
                            lhsT=pT[:, s, ts(sub, P)],
                            rhs=v_sb[:, m, 2 * dc + s, :],
                            start=(m == 0 and sub == 0),
                            stop=(m == MT - 1),
                            skip_group_check=True)

            for dc in range(2):
                for j in range(NJ):
                    oPs = [ps_o.tile([P, 4, DH + 1], F32, name=f"o{s}")
                           for s in range(2)]
                    pTs = []
                    for m in range(MT):
                        sT = ps_s.tile([P, 2, FD], F32, name="sT")
                        qk(sT, dc, j, m)
                        if m >= 2:
                            av(oPs, pTs[m - 2], dc, m - 2)
                        pT = pTp.tile([P, 2, FD], BF16, name="pT")
                        if m % 3 == 2:
                            # DVE fast-exp (Schraudolph) to offload ScalarE
                            it = i32p.tile([P, 2, FD], I32, name="it")
                            nc.vector.tensor_scalar(
                                it, sT, KS, BS,
                                op0=mybir.AluOpType.mult,
                                op1=mybir.AluOpType.add)
                            nc.vector.tensor_copy(pT, it.bitcast(F32))
                        else:
                            nc.scalar.activation(pT, sT, EXP, scale=SCALE)
                        pTs.append(pT)
                    av(oPs, pTs[MT - 2], dc, MT - 2)
                    av(oPs, pTs[MT - 1], dc, MT - 1)
                    # drain + normalize: D is column 64 of each (s, sub) block
                    o_sb = drn.tile([P, 2, 4, DH + 1], F32, name="o_sb")
                    for s in range(2):
                        nc.vector.tensor_copy(o_sb[:, s], oPs[s])
                    rc = norm.tile([P, 2, 4, 1], F32, name="rc")
                    nc.vector.reciprocal(rc, o_sb[:, :, :, DH:DH + 1])
                    o_bf = drn.tile([P, 4, 2, DH], BF16, name="o_bf")
                    for s in range(2):
                        for sub in range(4):
                            nc.vector.tensor_scalar_mul(
                                o_bf[:, sub, s, :], o_sb[:, s, sub, 0:DH],
                                rc[:, s, sub, :])
                    # transpose back to d-major for the output projection
                    if dc == 1 and j == NJ - 1:
                        # last group: deferred PE transpose (runs after the
                        # already-ready y tiles so the PE queue never stalls)
                        last_obf = o_bf
                    else:
                        for sub in range(4):
                            nc.sync.dma_start_transpose(
                                out=oTn[:, dc, j * FD + sub * P:
                                        j * FD + (sub + 1) * P],
                                in_=o_bf[:, sub].rearrange("p s d -> p (s d)"))

            ps_o_cm.__exit__(None, None, None)
            ps_s_cm.__exit__(None, None, None)

            # ---- phase C: output projection ----
            ps_y_cm = tc.tile_pool(name="ps_y", bufs=3, space="PSUM")
            ps_y = ps_y_cm.__enter__()

            def y_tile(i):
                y_ps = ps_y.tile([P, C], F32, name="y")
                for col in range(2):
                    for dc in range(2):
                        nc.tensor.matmul(
                            y_ps[:, ts(col, FD)],
                            lhsT=oTn[:, dc, ts(i, P)],
                            rhs=wo_sb[:, dc, ts(col, FD)],
                            start=(dc == 0), stop=(dc == 1))
                y_sb = yp.tile([P, C], BF16, name="ysb")
                nc.vector.tensor_copy(y_sb[:, 0:FD], y_ps[:, 0:FD])
                nc.scalar.copy(y_sb[:, FD:C], y_ps[:, FD:C])
                nc.sync.dma_start(out=y_d[ts(i, P), :], in_=y_sb)

            for i in range(NT - 4):
                y_tile(i)
            ps_t2_cm = tc.tile_pool(name="ps_t2", bufs=1, space="PSUM")
            ps_t2 = ps_t2_cm.__enter__()
            tp = ps_t2.tile([P, FD], F32, name="tpy")
            for sub in range(4):
                nc.tensor.matmul(
                    tp[:, ts(sub, P)],
                    lhsT=last_obf[:, sub].rearrange("p s d -> p (s d)"),
                    rhs=identb, start=True, stop=True)
            nc.vector.tensor_copy(oTn[:, 1, ts(NJ - 1, FD)], tp)
            for i in range(NT - 4, NT):
                y_tile(i)
            ps_t2_cm.__exit__(None, None, None)
            ps_y_cm.__exit__(None, None, None)

    nc.compile()
    return nc


def _in_maps(x, context, mask, Wq, Wk, Wv, Wo):
    from ml_dtypes import bfloat16
    maps = []
    xb = np.asarray(x, dtype=np.float32).astype(bfloat16)
    cb = np.asarray(context, dtype=np.float32).astype(bfloat16)
    for core in range(N_CORES):
        b, hg = core // H_PER, core % H_PER
        c0 = hg * DHC
        maps.append({
            "xT": np.ascontiguousarray(xb[b].T),
            "cT": np.ascontiguousarray(cb[b].T),
            "msk": np.ascontiguousarray(
                np.asarray(mask[b]).astype(np.float32).reshape(M, 1)),
            "wq": np.ascontiguousarray(
                np.asarray(Wq[:, c0:c0 + DHC], dtype=np.float32)
                .astype(bfloat16)),
            "wk": np.ascontiguousarray(
                np.asarray(Wk[:, c0:c0 + DHC], dtype=np.float32)
                .astype(bfloat16)),
            "wv": np.ascontiguousarray(
                np.asarray(Wv[:, c0:c0 + DHC], dtype=np.float32)
                .astype(bfloat16)),
            "wo": np.ascontiguousarray(
                np.asarray(Wo[c0:c0 + DHC, :], dtype=np.float32)
                .astype(bfloat16)),
        })
    return maps


def _gather(results, bo):
    out = np.zeros((B, N, C), dtype=np.float32)
    for core in range(N_CORES):
        out[core // H_PER] += np.asarray(results[core]["y"],
                                         dtype=np.float32)
    out += np.asarray(bo, dtype=np.float32)
    return out


def kernel(x, context, mask, Wq, Wk, Wv, Wo, bo, **extra_kwargs):
    if "nc" not in _CACHE:
        _CACHE["nc"] = _build()
    nc = _CACHE["nc"]
    maps = _in_maps(x, context, mask, Wq, Wk, Wv, Wo)
    res = run_bass_kernel_spmd(nc, maps, core_ids=list(range(N_CORES)),
                               **extra_kwargs)
    out = _gather(res.results, bo)
    if extra_kwargs:
        _CACHE["last_result"] = res
    return out
